# revision 39
# baseline (speedup 1.0000x reference)
"""Multi-head causal attention on 8 TRN2 NeuronCores.

Sharding: core c -> (b = c // 4, hg = c % 4). Data parallel over the batch
dim (B=2), tensor parallel over heads (16 heads -> 4 groups of 4). Each core
computes q/k/v projections for its 4 heads on its batch row, full causal
attention for those heads, and a partial output projection through its
256-row slice of Wp. The host sums the 4 head-group partials per batch
(the tensor-parallel reduce) and adds the output bias.

v2 changes vs the earlier 182us kernel:
- All inputs arrive pre-transposed and pre-cast to bf16 by the host shard
  step: x as per-stage [128, 8*512] xT blocks, weights in their SBUF
  layouts. This removes 128 PE transpose matmuls, all fp32->bf16 casts
  (DVE+ScalarE), and halves input DMA bytes.
- Score matmuls for the two heads of a qTc/kTc pair tile are emitted
  back-to-back: head A (partitions 0-63) and head B (64-127) auto-derive
  tile_position (0,0)/(64,0), so the two K=64 matmuls run concurrently in
  different PE row groups (~2x on the scores stream).
- One exp activation covers both heads' score banks (same diagonal offset),
  so the ScalarE fixed cost is amortized identically to before while the
  PE stream halves.
- q bias evac moved from ScalarE to DVE; the final out-proj stage (qc=3)
  is split per head-pair with SBUF accumulation so the kernel tail is only
  the pair-1 matmuls + one add-evac + store.
"""

import numpy as np
import ml_dtypes

import concourse.bass as bass
import concourse.mybir as mybir
import concourse.tile as tile
from concourse import bacc
from concourse.bass_utils import run_bass_kernel_spmd

F32 = mybir.dt.float32
BF16 = mybir.dt.bfloat16

B, T, C, H = 2, 2048, 1024, 16
NCORES = 8
HG = 4            # head groups (tensor-parallel degree)
NH = H // HG      # heads per core = 4
HD = C // H       # head dim = 64
HS = NH * HD      # head-slice width per core = 256
SCALE = 1.0 / float(np.sqrt(HD))

TB = T // 128     # 16 row blocks
CCH = C // 128    # 8 contraction chunks
QC = T // 512     # 4 q chunks of 512


def _body(tc):
    nc = tc.nc
    xt = nc.dram_tensor("xt", [QC, 128, CCH * 512], BF16, kind="ExternalInput").ap()
    wq = nc.dram_tensor("wq", [128, CCH * HS], BF16, kind="ExternalInput").ap()
    wk = nc.dram_tensor("wk", [128, CCH * HS], BF16, kind="ExternalInput").ap()
    wv = nc.dram_tensor("wv", [128, CCH * HS], BF16, kind="ExternalInput").ap()
    wp = nc.dram_tensor("wp", [128, (HS // 128) * C], BF16, kind="ExternalInput").ap()
    bq = nc.dram_tensor("bq", [128, 2], F32, kind="ExternalInput").ap()
    bk = nc.dram_tensor("bk", [128, 2], F32, kind="ExternalInput").ap()
    bv = nc.dram_tensor("bv", [1, HS], F32, kind="ExternalInput").ap()
    out = nc.dram_tensor("out", [T, C], BF16, kind="ExternalOutput").ap()

    with (
        tc.tile_pool(name="const", bufs=1) as const,
        tc.tile_pool(name="persist", bufs=1) as persist,
        tc.tile_pool(name="work", bufs=3) as work,
        tc.tile_pool(name="osbp", bufs=3) as osbp,
        tc.tile_pool(name="expp", bufs=6) as expp,
        tc.tile_pool(name="esb3p", bufs=1) as esb3p,
        tc.tile_pool(name="mmps", bufs=2, space="PSUM") as mmps,
        tc.tile_pool(name="sps2", bufs=2, space="PSUM") as spsp,
        tc.tile_pool(name="yps", bufs=2, space="PSUM") as ypsp,
    ):
        # ---- persistent SBUF tiles (DMA targets) ----------------------
        xT = [persist.tile([128, CCH, 512], BF16, tag=f"xT{t4}", name=f"xT{t4}")
              for t4 in range(QC)]
        wq_b = persist.tile([128, CCH, HS], BF16, tag="wq_b")
        wk_b = persist.tile([128, CCH, HS], BF16, tag="wk_b")
        wv_b = persist.tile([128, CCH, HS], BF16, tag="wv_b")
        wp_b = persist.tile([128, HS // 128, C], BF16, tag="wp_b")
        bq_sb = const.tile([128, 2], F32, tag="bq_sb")
        bk_sb = const.tile([128, 2], F32, tag="bk_sb")
        bv_row = const.tile([1, HS], F32, tag="bv_row")

        # ---- all input DMAs up front; queue order is load-bearing -----
        nc.sync.dma_start(xT[0][:], xt[0].rearrange("p (o t) -> p o t", t=512))
        nc.sync.dma_start(wq_b[:], wq.rearrange("p (o n) -> p o n", n=HS))
        nc.sync.dma_start(wk_b[:], wk.rearrange("p (o n) -> p o n", n=HS))
        nc.sync.dma_start(bq_sb[:], bq)
        nc.sync.dma_start(bk_sb[:], bk)
        nc.sync.dma_start(wv_b[:], wv.rearrange("p (o n) -> p o n", n=HS))
        nc.sync.dma_start(bv_row[:], bv)
        nc.sync.dma_start(xT[1][:], xt[1].rearrange("p (o t) -> p o t", t=512))
        nc.sync.dma_start(wp_b[:], wp.rearrange("p (o n) -> p o n", n=C))
        nc.sync.dma_start(xT[2][:], xt[2].rearrange("p (o t) -> p o t", t=512))
        nc.sync.dma_start(xT[3][:], xt[3].rearrange("p (o t) -> p o t", t=512))

        # ---- constants -------------------------------------------------
        # HAM warmup input: the memset must be gpsimd's first op
        warm_in = const.tile([128, 512], BF16, tag="warm_in")
        nc.gpsimd.memset(warm_in[:], 0.0)
        ones1 = const.tile([1, 128], BF16, tag="ones1")
        nc.gpsimd.memset(ones1[:], 1.0)
        # 0/1 lower-triangular mask (keep (i, j) iff j >= i) for the
        # diagonal 128-col strips, applied post-exp as a DVE multiply
        trimask = const.tile([128, 128], BF16, tag="trimask")
        nc.gpsimd.memset(trimask[:], 1.0)
        nc.gpsimd.affine_select(
            out=trimask[:], in_=trimask[:],
            compare_op=mybir.AluOpType.is_ge,
            fill=0.0, base=0, pattern=[[1, 128]], channel_multiplier=-1,
        )

        # v natural layout [T, 4 heads x (64 + ones col)]
        v_sb = [persist.tile([128, 4, NH * 65], BF16, tag=f"v_sb{i}",
                             name=f"v_sb{i}") for i in range(4)]
        for i in range(4):
            nc.gpsimd.memset(
                v_sb[i][:].rearrange("p k (h e) -> p k h e", e=65)[:, :, :, 64:65], 1.0
            )

        # initialize both sps ring slots so diagonal-unit exps may read
        # (and discard) the stale region left of the diagonal
        for r in range(2):
            sinit = spsp.tile([128, 2, 512], F32, tag="sps2", name=f"sinit{r}")
            nc.vector.memset(sinit[:], 0.0)

        # HAM warmup: full-K matmuls bridge the initial DMA window
        for r in range(15):
            wps = mmps.tile([128, 512], F32, tag="mm512", name=f"warm{r}")
            nc.tensor.matmul(wps[:], warm_in[:, :128], warm_in[:], start=True, stop=True)

        def dummy_mm(n, name):
            # keep-warm filler: the HAM clock gate re-throttles the PE to
            # 1.2 GHz after idle accumulation; a dummy matmul on the zero
            # tile keeps the activity window fed at ~100ns cost
            wps = mmps.tile([128, 512], F32, tag="mm512", name=name)
            nc.tensor.matmul(wps[:, :n], warm_in[:, :128], warm_in[:, :n],
                             start=True, stop=True)

        qTc = [[persist.tile([128, 512], BF16, tag=f"qTc{p}_{t}", name=f"qTc{p}_{t}")
                for t in range(QC)] for p in range(2)]
        kTc = [[persist.tile([128, 512], BF16, tag=f"kTc{p}_{t}", name=f"kTc{p}_{t}")
                for t in range(QC)] for p in range(2)]
        yT = [persist.tile([128, 512], BF16, tag=f"yT{q}", name=f"yT{q}")
              for q in range(QC * 2)]  # index 2*qc + pair

        def qk_one(t4, pair, which):
            w_b, b_sb = (wq_b, bq_sb) if which == "q" else (wk_b, bk_sb)
            dst = (qTc if which == "q" else kTc)[pair][t4]
            ps = mmps.tile([128, 512], F32, tag="mm512", name=f"{which}ps{pair}_{t4}")
            for cc in range(CCH):
                nc.tensor.matmul(
                    ps[:],
                    w_b[:, cc, pair * 128 : (pair + 1) * 128],
                    xT[t4][:, cc, :],
                    start=(cc == 0),
                    stop=(cc == CCH - 1),
                )
            nc.vector.tensor_scalar_add(dst[:], ps[:], b_sb[:, pair : pair + 1])

        def v_one(tb):
            ps = mmps.tile([128, 512], F32, tag="mm512", name=f"vps{tb}")
            for cc in range(CCH):
                nc.tensor.matmul(
                    ps[:, :HS],
                    xT[tb // 4][:, cc, (tb % 4) * 128 : (tb % 4 + 1) * 128],
                    wv_b[:, cc, :],
                    start=(cc == 0),
                    stop=(cc == CCH - 1),
                )
            vdst = v_sb[tb // 4][:, tb % 4, :].rearrange(
                "p (h e) -> p h e", e=65)[:, :, 0:64]
            nc.vector.tensor_tensor(vdst, ps[:, :HS], bv_bc[:], mybir.AluOpType.add)

        # ---- attention units: (pair, qc, kb), heads 2p/2p+1 packed ----
        units = []
        for qc in range(QC):
            for pair in range(2):
                nkb = 4 * qc + 4
                for kb in range(nkb):
                    units.append((pair, qc, kb, kb == nkb - 1))
        esbs = {}
        emitted = set()
        yps_tiles = {}

        def emit_scores(i, early=False):
            # scores + exp for both heads of the pair at this kb; the two
            # K=64 matmuls target row groups 0/64 and run concurrently.
            # early=True computes a qc=3 unit ahead of its phase (during
            # qc0/qc1, where ScalarE has slack) into a persistent esb.
            pair, qc, kb, _ = units[i]
            d = max(0, 128 * (kb - 4 * qc))
            sps = spsp.tile([128, 2, 512], F32, tag="sps2", name=f"sps{i}")
            if early:
                esb = esb3p.tile([128, 2, 512], BF16, tag=f"esb3_{i}",
                                 name=f"esb3_{i}")
            else:
                esb = expp.tile([128, 2, 512], BF16, tag="esb", name=f"esb{i}")
            kcol = slice((kb % 4) * 128, (kb % 4 + 1) * 128)
            for j in range(2):
                off = 64 * j
                nc.tensor.matmul(
                    sps[:, j, d:512],
                    kTc[pair][kb // 4][off : off + 64, kcol],
                    qTc[pair][qc][off : off + 64, d:512],
                    start=True, stop=True,
                )
            # one exp covers both heads' banks: for diagonal units the
            # range [d:1024] crosses bank1's never-computed cols [0:d)
            # (pre-memset, PV never reads them) -- a single ScalarE
            # dispatch saves the 300ns fixed cost of a second one
            flat_s = sps[:].rearrange("p a b -> p (a b)")
            flat_e = esb[:].rearrange("p a b -> p (a b)")
            nc.scalar.activation(
                flat_e[:, d:1024], flat_s[:, d:1024],
                mybir.ActivationFunctionType.Exp, scale=SCALE,
            )
            if kb >= 4 * qc:
                for j in range(2):
                    nc.vector.tensor_tensor(
                        esb[:, j, d : d + 128], esb[:, j, d : d + 128],
                        trimask[:], mybir.AluOpType.mult,
                    )
            esbs[i] = esb
            emitted.add(i)

        def normalize(h, qc):
            # row 64 of yps is the softmax denominator
            # (NOTE: reciprocal reading PSUM directly passes CoreSim but
            # yields garbage on hardware -- keep the SBUF copy)
            pair, off = h // 2, 64 * (h % 2)
            yps = yps_tiles.pop((h, qc))
            den = work.tile([1, 512], F32, tag="den")
            if qc == 3 and pair == 1:
                # kernel tail: the den copy rides the idle ScalarE so the
                # DVE only carries recip+mult on the critical chain
                nc.scalar.copy(den[:], yps[64:65, :])
            else:
                nc.vector.tensor_copy(den[:], yps[64:65, :])
            rec = work.tile([1, 512], F32, tag="rec")
            nc.vector.reciprocal_approx_fast(rec[:], den[:])
            rbc = work.tile([64, 512], F32, tag="rbc")
            nc.gpsimd.partition_broadcast(rbc[:], rec[:])
            nc.vector.tensor_tensor(
                yT[2 * qc + pair][off : off + 64, :],
                yps[0:64, :], rbc[:], mybir.AluOpType.mult,
            )

        def emit_pv(i):
            pair, qc, kb, is_last = units[i]
            d = max(0, 128 * (kb - 4 * qc))
            esb = esbs.pop(i)
            for j in range(2):
                h = 2 * pair + j
                if kb == 0:
                    yps_tiles[(h, qc)] = ypsp.tile(
                        [65, 512], F32, tag="yps", name=f"yps{h}_{qc}"
                    )
                nc.tensor.matmul(
                    yps_tiles[(h, qc)][:, d:512],
                    v_sb[kb // 4][:, kb % 4, 65 * h : 65 * h + 65],
                    esb[:, j, d:512],
                    start=(kb == 0),
                    stop=is_last,
                )
            if is_last:
                for j in range(2):
                    normalize(2 * pair + j, qc)

        # ---- output projection ----------------------------------------
        def emit_s4_qb(qc, qb, split_evac=False):
            # both pairs accumulated in PSUM; at the kernel tail the two
            # evac copies split across DVE and the (idle) ScalarE
            osb = osbp.tile([128, C], BF16, tag="osb", name=f"osb{qb}")
            for cc2 in range(2):
                ps = mmps.tile([128, 512], F32, tag="mm512", name=f"ops{qb}_{cc2}")
                for ych in range(HS // 128):
                    nc.tensor.matmul(
                        ps[:],
                        yT[2 * qc + ych][:, (qb % 4) * 128 : (qb % 4 + 1) * 128],
                        wp_b[:, ych, cc2 * 512 : (cc2 + 1) * 512],
                        start=(ych == 0),
                        stop=(ych == HS // 128 - 1),
                    )
                dst = osb[:, cc2 * 512 : (cc2 + 1) * 512]
                if split_evac and cc2 == 1:
                    nc.scalar.copy(dst, ps[:])
                else:
                    nc.vector.tensor_copy(dst, ps[:])
            nc.sync.dma_start(out[qb * 128 : (qb + 1) * 128, :], osb[:])

        # ---- prologue: stage-0 projections -----------------------------
        for pair in range(2):
            qk_one(0, pair, "q")
        for pair in range(2):
            qk_one(0, pair, "k")
        bv_rowb = const.tile([1, HS], BF16, tag="bv_rowb")
        nc.vector.tensor_copy(bv_rowb[:], bv_row[:])
        bv_bc = persist.tile([128, HS], F32, tag="bv_bc")
        ps = mmps.tile([128, 512], F32, tag="mm512")
        nc.tensor.matmul(ps[:, :HS], ones1[:], bv_rowb[:], start=True, stop=True)
        nc.vector.tensor_copy(bv_bc[:], ps[:, :HS])

        # ---- software-pipelined walk ----------------------------------
        # scores run LOOKAHEAD units ahead of PV within each (qc, pair)
        # segment; the segment cap quiesces the engines at boundaries
        # (letting the lookahead cross was measured slower: the denser
        # overlap stretched every exp/matmul via PSUM port contention)
        LOOKAHEAD = 4
        scores_done = 0
        build_steps = []

        def advance_scores(target, cap):
            nonlocal scores_done
            while scores_done < min(target, cap):
                if scores_done not in emitted:
                    emit_scores(scores_done)
                scores_done += 1

        def walk(lo, hi, pops):
            for i in range(lo, hi):
                # drips BEFORE the PV so just-in-time v blocks land ahead
                # of the PV matmul that reads them
                popped = 0
                while build_steps and popped < pops:
                    build_steps.pop(0)()
                    popped += 1
                if not popped and i >= 8:
                    # late phases have more ScalarE (exp) work than PE
                    # work per unit; filler keeps the HAM window fed
                    dummy_mm(256, f"fill{i}")
                advance_scores(i + 1 + LOOKAHEAD, hi)
                emit_pv(i)

        # drips per segment. v blocks are just-in-time (block tb is first
        # read by PV at kb=tb); stage t4's qk runs a phase early; qk for
        # stage 3 runs in qc0/qc1 so the first 8 key blocks of qc=3's
        # scores+exp (16 units) can ALSO run there -- qc0/qc1 have
        # ScalarE slack while qc=3 is exp-bound. Their probabilities wait
        # in persistent esb3 tiles; qc=3 keeps only the PV matmuls.
        steps_at = {
            0: [lambda tb=tb: v_one(tb) for tb in (0, 1, 2, 3)]
               + [lambda p=p, w=w: qk_one(1, p, w)
                  for p, w in ((0, "q"), (0, "k"), (1, "q"), (1, "k"))],
            8: [lambda tb=tb: v_one(tb) for tb in (4, 5, 6, 7)]
               + [lambda p=p, w=w: qk_one(2, p, w)
                  for p, w in ((0, "q"), (0, "k"), (1, "q"), (1, "k"))]
               + [lambda qb=qb: emit_s4_qb(0, qb) for qb in range(0, 2)],
            24: [lambda tb=tb: v_one(tb) for tb in (8, 9, 10, 11)]
                + [lambda w=w: qk_one(3, 0, w) for w in ("q", "k")]
                + [lambda qb=qb: emit_s4_qb(0, qb) for qb in range(2, 4)]
                + [lambda qb=qb: emit_s4_qb(1, qb) for qb in range(4, 6)]
                + [lambda w=w: qk_one(3, 1, w) for w in ("q", "k")]
                + [lambda u=u: emit_scores(u, early=True)
                   for u in (64, 65, 66, 67)],
            48: [lambda tb=tb: v_one(tb) for tb in (12, 13, 14, 15)]
                + [lambda qb=qb: emit_s4_qb(1, qb) for qb in range(6, 8)]
                + [lambda qb=qb: emit_s4_qb(2, qb) for qb in range(8, 10)],
            64: [lambda qb=qb: emit_s4_qb(2, qb) for qb in range(10, 12)],
        }

        seg_bounds = []
        lo = 0
        for qc in range(QC):
            n_pair = 4 * qc + 4
            seg_bounds += [(lo, lo + n_pair), (lo + n_pair, lo + 2 * n_pair)]
            lo += 2 * n_pair
        for lo, hi in seg_bounds:
            if lo in steps_at:
                build_steps.extend(steps_at[lo])
            walk(lo, hi, pops=1)
        while build_steps:
            build_steps.pop(0)()
        # bridge the final normalize chain (~4.5us of DVE/gpsimd latency)
        # so the tail out-proj matmuls run at full clock
        for r in range(20):
            dummy_mm(256, f"tailwarm{r}")
        for qb in range(12, 16):
            emit_s4_qb(3, qb, split_evac=True)


_NC = None


def _build():
    global _NC
    if _NC is None:
        nc = bacc.Bacc("TRN2", target_bir_lowering=False)
        with tile.TileContext(nc) as tc:
            _body(tc)
        nc.compile()
        _NC = nc
    return _NC


def _shard_inputs(x, Wq, bq, Wk, bk, Wv, bv, Wp, bp):
    bf16 = ml_dtypes.bfloat16
    f = lambda a: np.asarray(a, dtype=np.float32)

    def wtile(w, cols):
        # [1024, 256] slice -> SBUF layout [128, 8*256] bf16
        ws = f(w)[:, cols].reshape(CCH, 128, HS).transpose(1, 0, 2)
        return np.ascontiguousarray(ws.reshape(128, CCH * HS).astype(bf16))

    def ptile(w, cols):
        # [256, 1024] slice -> SBUF layout [128, 2*1024] bf16
        ws = f(w)[cols, :].reshape(HS // 128, 128, C).transpose(1, 0, 2)
        return np.ascontiguousarray(ws.reshape(128, (HS // 128) * C).astype(bf16))

    def btile(b, cols):
        return np.ascontiguousarray(f(b)[cols].reshape(2, 128).T)

    in_maps = []
    for c in range(NCORES):
        b, hg = divmod(c, HG)
        cols = slice(hg * HS, (hg + 1) * HS)
        # x[b].T split into 4 column-stages, each contiguous [128, 8*512]
        xtb = f(x)[b].T.reshape(CCH, 128, QC, 512).transpose(2, 1, 0, 3)
        xtb = np.ascontiguousarray(xtb.reshape(QC, 128, CCH * 512).astype(bf16))
        in_maps.append({
            "xt": xtb,
            "wq": wtile(Wq, cols), "wk": wtile(Wk, cols), "wv": wtile(Wv, cols),
            "wp": ptile(Wp, cols),
            "bq": btile(bq, cols), "bk": btile(bk, cols),
            "bv": np.ascontiguousarray(f(bv)[cols].reshape(1, HS)),
        })
    return in_maps


def run_sharded(inputs, **run_kwargs):
    """Compile (cached), run on cores 0-7, gather. Returns (out, results)."""
    nc = _build()
    in_maps = _shard_inputs(**inputs)
    res = run_bass_kernel_spmd(nc, in_maps, core_ids=list(range(NCORES)), **run_kwargs)
    out = np.zeros((B, T, C), np.float32)
    for c in range(NCORES):
        b = c // HG
        out[b] += res.results[c]["out"]
    out += np.asarray(inputs["bp"], dtype=np.float32)
    return out, res


def kernel(x, Wq, bq, Wk, bk, Wv, bv, Wp, bp):
    out, _ = run_sharded(dict(
        x=x, Wq=Wq, bq=bq, Wk=Wk, bk=bk, Wv=Wv, bv=bv, Wp=Wp, bp=bp,
    ))
    return out


# revision 43
# speedup vs baseline: 1.0039x; 1.0039x over previous
"""Multi-head causal attention on 8 TRN2 NeuronCores.

Sharding: core c -> (b = c // 4, hg = c % 4). Data parallel over the batch
dim (B=2), tensor parallel over heads (16 heads -> 4 groups of 4). Each core
computes q/k/v projections for its 4 heads on its batch row, full causal
attention for those heads, and a partial output projection through its
256-row slice of Wp. The host sums the 4 head-group partials per batch
(the tensor-parallel reduce) and adds the output bias.

v2 changes vs the earlier 182us kernel:
- All inputs arrive pre-transposed and pre-cast to bf16 by the host shard
  step: x as per-stage [128, 8*512] xT blocks, weights in their SBUF
  layouts. This removes 128 PE transpose matmuls, all fp32->bf16 casts
  (DVE+ScalarE), and halves input DMA bytes.
- Score matmuls for the two heads of a qTc/kTc pair tile are emitted
  back-to-back: head A (partitions 0-63) and head B (64-127) auto-derive
  tile_position (0,0)/(64,0), so the two K=64 matmuls run concurrently in
  different PE row groups (~2x on the scores stream).
- One exp activation covers both heads' score banks (same diagonal offset),
  so the ScalarE fixed cost is amortized identically to before while the
  PE stream halves.
- q bias evac moved from ScalarE to DVE; output partials stored as bf16
  (host accumulates in fp32), halving store DMA.
- Keep-warm filler matmuls bridge every PE idle window (prologue DMA wait,
  exp-paced stretches of qc2/qc3, and the final softmax-normalize chain):
  the HAM clock gate re-throttles the PE to 1.2 GHz after ~3.4us of idle
  accumulation, and each cold window costs ~2x on whatever follows.
- The per-(qc, pair) segment lookahead cap is load-bearing: letting the
  scores lookahead cross segment boundaries (or packing more work into
  the early phases) measurably stretches every exp/matmul duration --
  denser cross-engine overlap loses to PSUM/SBUF port contention here.
"""

import numpy as np
import ml_dtypes

import concourse.bass as bass
import concourse.mybir as mybir
import concourse.tile as tile
from concourse import bacc
from concourse.bass_utils import run_bass_kernel_spmd

F32 = mybir.dt.float32
BF16 = mybir.dt.bfloat16

B, T, C, H = 2, 2048, 1024, 16
NCORES = 8
HG = 4            # head groups (tensor-parallel degree)
NH = H // HG      # heads per core = 4
HD = C // H       # head dim = 64
HS = NH * HD      # head-slice width per core = 256
SCALE = 1.0 / float(np.sqrt(HD))

TB = T // 128     # 16 row blocks
CCH = C // 128    # 8 contraction chunks
QC = T // 512     # 4 q chunks of 512


def _body(tc):
    nc = tc.nc
    xt = nc.dram_tensor("xt", [QC, 128, CCH * 512], BF16, kind="ExternalInput").ap()
    wq = nc.dram_tensor("wq", [128, CCH * HS], BF16, kind="ExternalInput").ap()
    wk = nc.dram_tensor("wk", [128, CCH * HS], BF16, kind="ExternalInput").ap()
    wv = nc.dram_tensor("wv", [128, CCH * HS], BF16, kind="ExternalInput").ap()
    wp = nc.dram_tensor("wp", [128, (HS // 128) * C], BF16, kind="ExternalInput").ap()
    bq = nc.dram_tensor("bq", [128, 2], F32, kind="ExternalInput").ap()
    bk = nc.dram_tensor("bk", [128, 2], F32, kind="ExternalInput").ap()
    bv = nc.dram_tensor("bv", [1, HS], F32, kind="ExternalInput").ap()
    out = nc.dram_tensor("out", [T, C], BF16, kind="ExternalOutput").ap()

    with (
        tc.tile_pool(name="const", bufs=1) as const,
        tc.tile_pool(name="persist", bufs=1) as persist,
        tc.tile_pool(name="work", bufs=3) as work,
        tc.tile_pool(name="osbp", bufs=3) as osbp,
        tc.tile_pool(name="expp", bufs=6) as expp,
        tc.tile_pool(name="esb3p", bufs=1) as esb3p,
        tc.tile_pool(name="mmps", bufs=2, space="PSUM") as mmps,
        tc.tile_pool(name="sps2", bufs=2, space="PSUM") as spsp,
        tc.tile_pool(name="yps", bufs=2, space="PSUM") as ypsp,
    ):
        # ---- persistent SBUF tiles (DMA targets) ----------------------
        xT = [persist.tile([128, CCH, 512], BF16, tag=f"xT{t4}", name=f"xT{t4}")
              for t4 in range(QC)]
        wq_b = persist.tile([128, CCH, HS], BF16, tag="wq_b")
        wk_b = persist.tile([128, CCH, HS], BF16, tag="wk_b")
        wv_b = persist.tile([128, CCH, HS], BF16, tag="wv_b")
        wp_b = persist.tile([128, HS // 128, C], BF16, tag="wp_b")
        bq_sb = const.tile([128, 2], F32, tag="bq_sb")
        bk_sb = const.tile([128, 2], F32, tag="bk_sb")
        bv_row = const.tile([1, HS], F32, tag="bv_row")

        # ---- all input DMAs up front; queue order is load-bearing -----
        nc.sync.dma_start(xT[0][:], xt[0].rearrange("p (o t) -> p o t", t=512))
        nc.sync.dma_start(wq_b[:], wq.rearrange("p (o n) -> p o n", n=HS))
        nc.sync.dma_start(wk_b[:], wk.rearrange("p (o n) -> p o n", n=HS))
        nc.sync.dma_start(bq_sb[:], bq)
        nc.sync.dma_start(bk_sb[:], bk)
        nc.sync.dma_start(wv_b[:], wv.rearrange("p (o n) -> p o n", n=HS))
        nc.sync.dma_start(bv_row[:], bv)
        nc.sync.dma_start(xT[1][:], xt[1].rearrange("p (o t) -> p o t", t=512))
        nc.sync.dma_start(wp_b[:], wp.rearrange("p (o n) -> p o n", n=C))
        nc.sync.dma_start(xT[2][:], xt[2].rearrange("p (o t) -> p o t", t=512))
        nc.sync.dma_start(xT[3][:], xt[3].rearrange("p (o t) -> p o t", t=512))

        # ---- constants -------------------------------------------------
        # HAM warmup input: the memset must be gpsimd's first op
        warm_in = const.tile([128, 512], BF16, tag="warm_in")
        nc.gpsimd.memset(warm_in[:], 0.0)
        ones1 = const.tile([1, 128], BF16, tag="ones1")
        nc.gpsimd.memset(ones1[:], 1.0)
        # 0/1 lower-triangular mask (keep (i, j) iff j >= i) for the
        # diagonal 128-col strips, applied post-exp as a DVE multiply
        trimask = const.tile([128, 128], BF16, tag="trimask")
        nc.gpsimd.memset(trimask[:], 1.0)
        nc.gpsimd.affine_select(
            out=trimask[:], in_=trimask[:],
            compare_op=mybir.AluOpType.is_ge,
            fill=0.0, base=0, pattern=[[1, 128]], channel_multiplier=-1,
        )

        # v natural layout [T, 4 heads x (64 + ones col)]
        v_sb = [persist.tile([128, 4, NH * 65], BF16, tag=f"v_sb{i}",
                             name=f"v_sb{i}") for i in range(4)]
        for i in range(4):
            nc.gpsimd.memset(
                v_sb[i][:].rearrange("p k (h e) -> p k h e", e=65)[:, :, :, 64:65], 1.0
            )

        # HAM warmup: full-K matmuls bridge the initial DMA window
        for r in range(15):
            wps = mmps.tile([128, 512], F32, tag="mm512", name=f"warm{r}")
            nc.tensor.matmul(wps[:], warm_in[:, :128], warm_in[:], start=True, stop=True)

        def dummy_mm(n, name):
            # keep-warm filler: the HAM clock gate re-throttles the PE to
            # 1.2 GHz after idle accumulation; a dummy matmul on the zero
            # tile keeps the activity window fed at ~100ns cost
            wps = mmps.tile([128, 512], F32, tag="mm512", name=name)
            nc.tensor.matmul(wps[:, :n], warm_in[:, :128], warm_in[:, :n],
                             start=True, stop=True)

        qTc = [[persist.tile([128, 512], BF16, tag=f"qTc{p}_{t}", name=f"qTc{p}_{t}")
                for t in range(QC)] for p in range(2)]
        kTc = [[persist.tile([128, 512], BF16, tag=f"kTc{p}_{t}", name=f"kTc{p}_{t}")
                for t in range(QC)] for p in range(2)]
        yT = [persist.tile([128, 512], BF16, tag=f"yT{q}", name=f"yT{q}")
              for q in range(QC * 2)]  # index 2*qc + pair

        def qk_one(t4, pair, which):
            w_b, b_sb = (wq_b, bq_sb) if which == "q" else (wk_b, bk_sb)
            dst = (qTc if which == "q" else kTc)[pair][t4]
            ps = mmps.tile([128, 512], F32, tag="mm512", name=f"{which}ps{pair}_{t4}")
            for cc in range(CCH):
                nc.tensor.matmul(
                    ps[:],
                    w_b[:, cc, pair * 128 : (pair + 1) * 128],
                    xT[t4][:, cc, :],
                    start=(cc == 0),
                    stop=(cc == CCH - 1),
                )
            nc.vector.tensor_scalar_add(dst[:], ps[:], b_sb[:, pair : pair + 1])

        def v_one(tb):
            ps = mmps.tile([128, 512], F32, tag="mm512", name=f"vps{tb}")
            for cc in range(CCH):
                nc.tensor.matmul(
                    ps[:, :HS],
                    xT[tb // 4][:, cc, (tb % 4) * 128 : (tb % 4 + 1) * 128],
                    wv_b[:, cc, :],
                    start=(cc == 0),
                    stop=(cc == CCH - 1),
                )
            vdst = v_sb[tb // 4][:, tb % 4, :].rearrange(
                "p (h e) -> p h e", e=65)[:, :, 0:64]
            nc.vector.tensor_tensor(vdst, ps[:, :HS], bv_bc[:], mybir.AluOpType.add)

        # ---- attention units: (pair, qc, kb), heads 2p/2p+1 packed ----
        units = []
        for qc in range(QC):
            for pair in range(2):
                nkb = 4 * qc + 4
                for kb in range(nkb):
                    units.append((pair, qc, kb, kb == nkb - 1))
        esbs = {}
        emitted = set()
        yps_tiles = {}

        def emit_scores(i, early=False):
            # scores + exp for both heads of the pair at this kb; the two
            # K=64 matmuls target row groups 0/64 and run concurrently.
            # early=True computes a qc=3 unit ahead of its phase (during
            # qc0/qc1, where ScalarE has slack) into a persistent esb.
            pair, qc, kb, _ = units[i]
            d = max(0, 128 * (kb - 4 * qc))
            sps = spsp.tile([128, 2, 512], F32, tag="sps2", name=f"sps{i}")
            if early:
                esb = esb3p.tile([128, 2, 512], BF16, tag=f"esb3_{i}",
                                 name=f"esb3_{i}")
            else:
                esb = expp.tile([128, 2, 512], BF16, tag="esb", name=f"esb{i}")
            kcol = slice((kb % 4) * 128, (kb % 4 + 1) * 128)
            for j in range(2):
                off = 64 * j
                nc.tensor.matmul(
                    sps[:, j, d:512],
                    kTc[pair][kb // 4][off : off + 64, kcol],
                    qTc[pair][qc][off : off + 64, d:512],
                    start=True, stop=True,
                )
            # one exp covers both heads when the unit is off-diagonal;
            # diagonal units split in two so no unwritten PSUM is read
            flat_s = sps[:].rearrange("p a b -> p (a b)")
            flat_e = esb[:].rearrange("p a b -> p (a b)")
            if d == 0:
                nc.scalar.activation(
                    flat_e[:, 0:1024], flat_s[:, 0:1024],
                    mybir.ActivationFunctionType.Exp, scale=SCALE,
                )
            else:
                nc.scalar.activation(
                    flat_e[:, d:512], flat_s[:, d:512],
                    mybir.ActivationFunctionType.Exp, scale=SCALE,
                )
                nc.scalar.activation(
                    flat_e[:, 512 + d : 1024], flat_s[:, 512 + d : 1024],
                    mybir.ActivationFunctionType.Exp, scale=SCALE,
                )
            if kb >= 4 * qc:
                for j in range(2):
                    nc.vector.tensor_tensor(
                        esb[:, j, d : d + 128], esb[:, j, d : d + 128],
                        trimask[:], mybir.AluOpType.mult,
                    )
            esbs[i] = esb
            emitted.add(i)

        def normalize(h, qc):
            # row 64 of yps is the softmax denominator
            # (NOTE: reciprocal reading PSUM directly passes CoreSim but
            # yields garbage on hardware -- keep the SBUF copy)
            pair, off = h // 2, 64 * (h % 2)
            yps = yps_tiles.pop((h, qc))
            den = work.tile([1, 512], F32, tag="den")
            if qc == 3 and pair == 1:
                # kernel tail: the den copy rides the idle ScalarE so the
                # DVE only carries recip+mult on the critical chain
                nc.scalar.copy(den[:], yps[64:65, :])
            else:
                nc.vector.tensor_copy(den[:], yps[64:65, :])
            rec = work.tile([1, 512], F32, tag="rec")
            nc.vector.reciprocal_approx_fast(rec[:], den[:])
            rbc = work.tile([64, 512], F32, tag="rbc")
            nc.gpsimd.partition_broadcast(rbc[:], rec[:])
            nc.vector.tensor_tensor(
                yT[2 * qc + pair][off : off + 64, :],
                yps[0:64, :], rbc[:], mybir.AluOpType.mult,
            )

        def emit_pv(i):
            pair, qc, kb, is_last = units[i]
            d = max(0, 128 * (kb - 4 * qc))
            esb = esbs.pop(i)
            for j in range(2):
                h = 2 * pair + j
                if kb == 0:
                    yps_tiles[(h, qc)] = ypsp.tile(
                        [65, 512], F32, tag="yps", name=f"yps{h}_{qc}"
                    )
                nc.tensor.matmul(
                    yps_tiles[(h, qc)][:, d:512],
                    v_sb[kb // 4][:, kb % 4, 65 * h : 65 * h + 65],
                    esb[:, j, d:512],
                    start=(kb == 0),
                    stop=is_last,
                )
            if is_last:
                for j in range(2):
                    normalize(2 * pair + j, qc)

        # ---- output projection ----------------------------------------
        def emit_s4_qb(qc, qb, split_evac=False):
            # both pairs accumulated in PSUM; at the kernel tail the two
            # evac copies split across DVE and the (idle) ScalarE
            osb = osbp.tile([128, C], BF16, tag="osb", name=f"osb{qb}")
            for cc2 in range(2):
                ps = mmps.tile([128, 512], F32, tag="mm512", name=f"ops{qb}_{cc2}")
                for ych in range(HS // 128):
                    nc.tensor.matmul(
                        ps[:],
                        yT[2 * qc + ych][:, (qb % 4) * 128 : (qb % 4 + 1) * 128],
                        wp_b[:, ych, cc2 * 512 : (cc2 + 1) * 512],
                        start=(ych == 0),
                        stop=(ych == HS // 128 - 1),
                    )
                dst = osb[:, cc2 * 512 : (cc2 + 1) * 512]
                if split_evac and cc2 == 1:
                    nc.scalar.copy(dst, ps[:])
                else:
                    nc.vector.tensor_copy(dst, ps[:])
            nc.sync.dma_start(out[qb * 128 : (qb + 1) * 128, :], osb[:])

        # ---- prologue: stage-0 projections -----------------------------
        for pair in range(2):
            qk_one(0, pair, "q")
        for pair in range(2):
            qk_one(0, pair, "k")
        bv_rowb = const.tile([1, HS], BF16, tag="bv_rowb")
        nc.vector.tensor_copy(bv_rowb[:], bv_row[:])
        bv_bc = persist.tile([128, HS], F32, tag="bv_bc")
        ps = mmps.tile([128, 512], F32, tag="mm512")
        nc.tensor.matmul(ps[:, :HS], ones1[:], bv_rowb[:], start=True, stop=True)
        nc.vector.tensor_copy(bv_bc[:], ps[:, :HS])

        # ---- software-pipelined walk ----------------------------------
        # scores run LOOKAHEAD units ahead of PV within each (qc, pair)
        # segment; the segment cap quiesces the engines at boundaries
        # (letting the lookahead cross was measured slower: the denser
        # overlap stretched every exp/matmul via PSUM port contention)
        LOOKAHEAD = 4
        scores_done = 0
        build_steps = []

        def advance_scores(target, cap):
            nonlocal scores_done
            while scores_done < min(target, cap):
                if scores_done not in emitted:
                    emit_scores(scores_done)
                scores_done += 1

        def walk(lo, hi, pops):
            for i in range(lo, hi):
                # drips BEFORE the PV so just-in-time v blocks land ahead
                # of the PV matmul that reads them
                popped = 0
                while build_steps and popped < pops:
                    build_steps.pop(0)()
                    popped += 1
                if not popped and i >= 8:
                    # late phases have more ScalarE (exp) work than PE
                    # work per unit; filler keeps the HAM window fed
                    dummy_mm(256, f"fill{i}")
                advance_scores(i + 1 + LOOKAHEAD, hi)
                emit_pv(i)

        # drips per segment. v blocks are just-in-time (block tb is first
        # read by PV at kb=tb); stage t4's qk runs a phase early; qk for
        # stage 3 runs in qc0/qc1 so the first 8 key blocks of qc=3's
        # scores+exp (16 units) can ALSO run there -- qc0/qc1 have
        # ScalarE slack while qc=3 is exp-bound. Their probabilities wait
        # in persistent esb3 tiles; qc=3 keeps only the PV matmuls.
        steps_at = {
            0: [lambda tb=tb: v_one(tb) for tb in (0, 1, 2, 3)]
               + [lambda p=p, w=w: qk_one(1, p, w)
                  for p, w in ((0, "q"), (0, "k"), (1, "q"), (1, "k"))],
            8: [lambda tb=tb: v_one(tb) for tb in (4, 5, 6, 7)]
               + [lambda p=p, w=w: qk_one(2, p, w)
                  for p, w in ((0, "q"), (0, "k"), (1, "q"), (1, "k"))]
               + [lambda qb=qb: emit_s4_qb(0, qb) for qb in range(0, 2)],
            24: [lambda tb=tb: v_one(tb) for tb in (8, 9, 10, 11)]
                + [lambda w=w: qk_one(3, 0, w) for w in ("q", "k")]
                + [lambda qb=qb: emit_s4_qb(0, qb) for qb in range(2, 4)]
                + [lambda qb=qb: emit_s4_qb(1, qb) for qb in range(4, 6)],
            48: [lambda tb=tb: v_one(tb) for tb in (12, 13, 14, 15)]
                + [lambda w=w: qk_one(3, 1, w) for w in ("q", "k")]
                + [lambda qb=qb: emit_s4_qb(1, qb) for qb in range(6, 8)]
                + [lambda qb=qb: emit_s4_qb(2, qb) for qb in range(8, 10)],
            64: [lambda qb=qb: emit_s4_qb(2, qb) for qb in range(10, 12)],
        }

        seg_bounds = []
        lo = 0
        for qc in range(QC):
            n_pair = 4 * qc + 4
            seg_bounds += [(lo, lo + n_pair), (lo + n_pair, lo + 2 * n_pair)]
            lo += 2 * n_pair
        for lo, hi in seg_bounds:
            if lo in steps_at:
                build_steps.extend(steps_at[lo])
            walk(lo, hi, pops=1)
        while build_steps:
            build_steps.pop(0)()
        # bridge the final normalize chain (~4.5us of DVE/gpsimd latency)
        # so the tail out-proj matmuls run at full clock
        for r in range(20):
            dummy_mm(256, f"tailwarm{r}")
        for qb in range(12, 16):
            emit_s4_qb(3, qb, split_evac=True)


_NC = None


def _build():
    global _NC
    if _NC is None:
        nc = bacc.Bacc("TRN2", target_bir_lowering=False)
        with tile.TileContext(nc) as tc:
            _body(tc)
        nc.compile()
        _NC = nc
    return _NC


def _shard_inputs(x, Wq, bq, Wk, bk, Wv, bv, Wp, bp):
    bf16 = ml_dtypes.bfloat16
    f = lambda a: np.asarray(a, dtype=np.float32)

    def wtile(w, cols):
        # [1024, 256] slice -> SBUF layout [128, 8*256] bf16
        ws = f(w)[:, cols].reshape(CCH, 128, HS).transpose(1, 0, 2)
        return np.ascontiguousarray(ws.reshape(128, CCH * HS).astype(bf16))

    def ptile(w, cols):
        # [256, 1024] slice -> SBUF layout [128, 2*1024] bf16
        ws = f(w)[cols, :].reshape(HS // 128, 128, C).transpose(1, 0, 2)
        return np.ascontiguousarray(ws.reshape(128, (HS // 128) * C).astype(bf16))

    def btile(b, cols):
        return np.ascontiguousarray(f(b)[cols].reshape(2, 128).T)

    in_maps = []
    for c in range(NCORES):
        b, hg = divmod(c, HG)
        cols = slice(hg * HS, (hg + 1) * HS)
        # x[b].T split into 4 column-stages, each contiguous [128, 8*512]
        xtb = f(x)[b].T.reshape(CCH, 128, QC, 512).transpose(2, 1, 0, 3)
        xtb = np.ascontiguousarray(xtb.reshape(QC, 128, CCH * 512).astype(bf16))
        in_maps.append({
            "xt": xtb,
            "wq": wtile(Wq, cols), "wk": wtile(Wk, cols), "wv": wtile(Wv, cols),
            "wp": ptile(Wp, cols),
            "bq": btile(bq, cols), "bk": btile(bk, cols),
            "bv": np.ascontiguousarray(f(bv)[cols].reshape(1, HS)),
        })
    return in_maps


def run_sharded(inputs, **run_kwargs):
    """Compile (cached), run on cores 0-7, gather. Returns (out, results)."""
    nc = _build()
    in_maps = _shard_inputs(**inputs)
    res = run_bass_kernel_spmd(nc, in_maps, core_ids=list(range(NCORES)), **run_kwargs)
    out = np.zeros((B, T, C), np.float32)
    for c in range(NCORES):
        b = c // HG
        out[b] += res.results[c]["out"]
    out += np.asarray(inputs["bp"], dtype=np.float32)
    return out, res


def kernel(x, Wq, bq, Wk, bk, Wv, bv, Wp, bp):
    out, _ = run_sharded(dict(
        x=x, Wq=Wq, bq=bq, Wk=Wk, bk=bk, Wv=Wv, bv=bv, Wp=Wp, bp=bp,
    ))
    return out
